# revision 10
# baseline (speedup 1.0000x reference)
"""Trainium2 Bass kernel for nn_AttnNet: attention-pooling over sequence (v7).

Reference computation (per batch b):
    act    = tanh(X @ W.T + b)          # [S, H]
    scores = act @ context              # [S]
    w      = exp(scores * mask)         # masked_fill(-1e-32) == *mask (exp(0)=1)
    out    = (X.T @ w) / sum(w)         # [H]

Sharding: pure data-parallel, 4 batches per core across 8 cores.

v7 vs v6 (175998 ns):
  * score reduction moved off the PE: the 128 replicated-ctx matmuls
    (27us of PE time) become 4 DVE tensor_scalar ops (4x fast mode,
    act*ctx per m-chunk) + 2 tensor_tensor pair-adds per half, finished
    by TWO accumulating ones-broadcast matmuls per subgroup (the second
    reduction level rides the PSUM accumulator in f32, which also keeps
    bf16 rounding out of the final sum).  PE work drops 136us -> ~123us.
  * act GEMM emits (glp, m) blocks of two 512-col subgroups into a
    2-bank PSUM tile so one activation instruction tanh's 1024 columns
    (Act engine 110us -> ~93us, same bias per-partition chunk).
  * pooling split across engines: k=0,1 on DVE, k=2,3 on GpSimd(Pool),
    which is otherwise idle -> DVE ~100us < PE.
  * startup ramp: xt b0 uploaded in 3 slices so the first MM starts at
    ~4us instead of 12us; wt/bias/ctx in 2 DMAs.
  * drain tail: the last half computes scores per gl-PAIR so pooling of
    its first half overlaps the remaining MM blocks.

Device layout (per core):
    xt   [BPC, KC, 128, S]  bf16  xt[b,k,p,s] = X[b, s, 128k+p]  (X^T)
    wt   [KC, 128, H]       bf16  wt[k,p,o]   = W[o, 128k+p]     (W^T)
    bc   [128, 2*MC] f32    bias (cols 0:MC) and context (cols MC:2MC),
                            bc[p, MC+m] = context[128m+p]
    mask [BPC, 128, S] bf16 (row-replicated across partitions)
outputs:
    num  [BPC, 128, KC, 3] f32  partial pooled sums (host combines)
    den  [BPC, NXT*GPH]    f32  partial denominators (host combines)
"""

import numpy as np
import ml_dtypes

import concourse.bass as bass
import concourse.tile as tile
from concourse import bacc, mybir
from concourse.bass_utils import run_bass_kernel_spmd

N_CORES = 8
B, S, H = 32, 4096, 512
BPC = B // N_CORES
P = 128
KC = H // P          # 4 contraction chunks
MC = H // P          # 4 output-channel chunks
SG = 512             # one PSUM bank of f32 columns
NXT = 2
HALF = S // NXT      # 2048
GPH = HALF // SG     # 4 subgroups per half
NSLOT = 3            # num accum slots: h0, h1/glp0, glp1

# pooling k-chunk routing: DVE runs stt (mult+accum) for k in DVE_KS; the
# remaining chunk is computed as a GpSimd tensor_tensor product and reduced
# by an Act-engine Copy+accum (Pool has no TensorScalarPtr in the ISA).
DVE_KS = (0, 1, 2)
PROD_K = 3

F32 = mybir.dt.float32
BF16 = mybir.dt.bfloat16
BF = ml_dtypes.bfloat16

TRACE = False
TRACE_DIR = None
LAST = {}


def build():
    nc = bacc.Bacc("TRN2", target_bir_lowering=False, num_devices=N_CORES)
    xt_d = nc.declare_dram_parameter("xt", [BPC, KC, P, S], BF16, isOutput=False)
    wt_d = nc.declare_dram_parameter("wt", [KC, P, H], BF16, isOutput=False)
    bc_d = nc.declare_dram_parameter("bc", [P, 2 * MC], F32, isOutput=False)
    mask_d = nc.declare_dram_parameter("mask", [BPC, P, S], BF16, isOutput=False)
    num_d = nc.declare_dram_parameter("num", [BPC, P, KC, NSLOT], F32, isOutput=True)
    den_d = nc.declare_dram_parameter("den", [BPC, NXT * GPH], F32, isOutput=True)

    Tanh = mybir.ActivationFunctionType.Tanh
    Exp = mybir.ActivationFunctionType.Exp
    Copy = mybir.ActivationFunctionType.Copy
    Mult = mybir.AluOpType.mult
    Add = mybir.AluOpType.add

    with tile.TileContext(nc) as tc:
        with (
            tc.tile_pool(name="singles", bufs=1) as singles,
            tc.tile_pool(name="xtp", bufs=2) as xtp,
            tc.tile_pool(name="actpool", bufs=2) as actpool,
            tc.tile_pool(name="maskpool", bufs=2) as maskpool,
            tc.tile_pool(name="tsp", bufs=1) as tsp,
            tc.tile_pool(name="saddp", bufs=2) as saddp,
            tc.tile_pool(name="efull", bufs=2) as efullp,
            tc.tile_pool(name="wbc", bufs=2) as wbcp,
            tc.tile_pool(name="trash", bufs=2) as trashp,
            tc.tile_pool(name="nums", bufs=2) as nums,
            tc.tile_pool(name="dens", bufs=2) as dens,
            tc.tile_pool(name="actps", bufs=3, space="PSUM") as actps,
            tc.tile_pool(name="scps", bufs=2, space="PSUM") as scps,
        ):
            halves = [(b, h) for b in range(BPC) for h in range(NXT)]
            NH = len(halves)

            xt_tiles = {}
            mask_tiles = {}
            num_tiles = {}
            den_tiles = {}
            act_tiles = {}    # per half
            sadd_tiles = {}   # per half: (s01, s23)
            wb_tiles = {}     # per half
            env = {}

            def load_xt(b):
                xt_sb = xtp.tile([P, KC, S], BF16, tag="xt", name="xt_sb")
                xt_tiles[b] = xt_sb
                src = xt_d.ap()[b].rearrange("k p s -> p k s")
                if b == 0:
                    # slice uploads so the first MM block starts early
                    nc.sync.dma_start(out=xt_sb[:, :, 0:1024], in_=src[:, :, 0:1024])
                    nc.sync.dma_start(
                        out=xt_sb[:, :, 1024:2048], in_=src[:, :, 1024:2048]
                    )
                    nc.sync.dma_start(out=xt_sb[:, :, 2048:], in_=src[:, :, 2048:])
                else:
                    nc.sync.dma_start(out=xt_sb[:, :, :], in_=src)

            def load_batch_state(b):
                mask_sb = maskpool.tile([P, S], BF16, tag="mask")
                mask_tiles[b] = mask_sb
                nc.sync.dma_start(out=mask_sb[:, :], in_=mask_d.ap()[b])
                num_tiles[b] = nums.tile([P, KC, NSLOT], F32, tag="num", name="num_sb")
                if b < BPC - 1:
                    nc.gpsimd.memset(num_tiles[b][:, :, 2:3], 0.0)
                den_tiles[b] = dens.tile([P, NXT * GPH], F32, tag="den", name="den_sb")

            def emit_ts(i, m):
                """t[m] = act[:, m, :, :] * ctx_col[m]  (DVE 4x)"""
                if i not in env.setdefault("ts", {}):
                    env["ts"][i] = {}
                t = tsp.tile([P, HALF], BF16, tag=f"ts{m}", name=f"ts{m}_t")
                env["ts"][i][m] = t
                nc.vector.tensor_scalar(
                    out=t[:, :],
                    in0=act_tiles[i][:, m, :, :],
                    scalar1=env["bc_sb"][:, MC + m : MC + m + 1],
                    scalar2=None,
                    op0=Mult,
                )

            def emit_sadd(i, pair):
                """s01 = t0 + t1 / s23 = t2 + t3  (on GpSimd: DVE relief)"""
                ts = env["ts"][i]
                s = saddp.tile([P, HALF], BF16, tag=f"sadd{pair}", name=f"s{pair}_t")
                if i not in sadd_tiles:
                    sadd_tiles[i] = {}
                sadd_tiles[i][pair] = s
                m0 = 2 * pair
                nc.gpsimd.tensor_tensor(
                    out=s[:, :], in0=ts[m0][:, :], in1=ts[m0 + 1][:, :], op=Add
                )

            def emit_scores(i, gl):
                """scores for subgroup gl of half i: two accumulating
                ones-broadcast MMs -> exp -> masked wb slice + den part."""
                b, h = halves[i]
                s01 = sadd_tiles[i][0]
                s23 = sadd_tiles[i][1]
                scp = scps.tile([P, SG], F32, tag="scp", name="scp_t")
                csl = slice(gl * SG, (gl + 1) * SG)
                nc.tensor.matmul(
                    scp[:, :], lhsT=env["ones"][:, :], rhs=s01[:, csl],
                    start=True, stop=False,
                )
                nc.tensor.matmul(
                    scp[:, :], lhsT=env["ones"][:, :], rhs=s23[:, csl],
                    start=False, stop=True,
                )
                ef = efullp.tile([P, SG], BF16, tag="ef", name="ef_t")
                nc.scalar.activation(out=ef[:, :], in_=scp[:, :], func=Exp)
                if gl == 0:
                    wb_tiles[i] = wbcp.tile([P, HALF], BF16, tag="wb", name="wb_t")
                wb = wb_tiles[i]
                ssl = slice(h * HALF + gl * SG, h * HALF + (gl + 1) * SG)
                nc.vector.scalar_tensor_tensor(
                    out=wb[:, csl],
                    in0=ef[:, :],
                    scalar=-1.0,
                    in1=mask_tiles[b][:, ssl],
                    op0=Add,
                    op1=Mult,
                    accum_out=den_tiles[b][:, h * GPH + gl : h * GPH + gl + 1],
                )

            def emit_pool(i, slot=None, cols=None):
                """pooling for half i: num[:, k, slot] += xt[k] . wb
                k in DVE_KS via DVE stt+accum; PROD_K via GpSimd product
                + Act Copy+accum reduction."""
                b, h = halves[i]
                xt_sb = xt_tiles[b]
                wb = wb_tiles[i]
                if slot is None:
                    slot = h
                c0, c1 = cols if cols is not None else (0, HALF)
                for k in DVE_KS:
                    trash = trashp.tile(
                        [P, HALF], BF16, tag=f"trash{k % 2}v", name="trash_t"
                    )
                    nc.vector.scalar_tensor_tensor(
                        out=trash[:, 0 : c1 - c0],
                        in0=xt_sb[:, k, h * HALF + c0 : h * HALF + c1],
                        scalar=1.0,
                        in1=wb[:, c0:c1],
                        op0=Mult,
                        op1=Mult,
                        accum_out=num_tiles[b][:, k, slot : slot + 1],
                    )
                k = PROD_K
                prod = trashp.tile([P, HALF], BF16, tag="prod", name="prod_t")
                nc.gpsimd.tensor_tensor(
                    out=prod[:, 0 : c1 - c0],
                    in0=xt_sb[:, k, h * HALF + c0 : h * HALF + c1],
                    in1=wb[:, c0:c1],
                    op=Mult,
                )
                trash = trashp.tile([P, HALF], BF16, tag="trashact", name="trash_t")
                nc.scalar.activation(
                    out=trash[:, 0 : c1 - c0],
                    in_=prod[:, 0 : c1 - c0],
                    func=Copy,
                    accum_out=num_tiles[b][:, k, slot : slot + 1],
                )

            def emit_out(b):
                nc.sync.dma_start(out=num_d.ap()[b], in_=num_tiles.pop(b)[:, :, :])
                nc.sync.dma_start(
                    out=den_d.ap()[b : b + 1, :], in_=den_tiles.pop(b)[0:1, :]
                )

            for i, (b, h) in enumerate(halves):
                last = i == NH - 1
                if i == 0:
                    load_xt(0)
                    wt_sb = singles.tile([P, KC, H], BF16)
                    nc.sync.dma_start(
                        out=wt_sb[:, :, :], in_=wt_d.ap().rearrange("k p h -> p k h")
                    )
                    bc_sb = singles.tile([P, 2 * MC], F32)
                    nc.sync.dma_start(out=bc_sb[:, :], in_=bc_d.ap())
                    env["bc_sb"] = bc_sb
                    ones = singles.tile([P, P], BF16)
                    nc.gpsimd.memset(ones[:, :], 1.0)
                    env["ones"] = ones
                if h == 0:
                    load_batch_state(b)

                act_sb = actpool.tile([P, MC, GPH, SG], BF16, tag="act")
                act_tiles[i] = act_sb
                xt_sb = xt_tiles[b]

                for glp in range(2):
                    for m in range(MC):
                        ps = actps.tile([P, 2, SG], F32, tag="ps")
                        for j in range(2):
                            gl = glp * 2 + j
                            ssl = slice(h * HALF + gl * SG, h * HALF + (gl + 1) * SG)
                            for k in range(KC):
                                nc.tensor.matmul(
                                    ps[:, j, :],
                                    lhsT=wt_sb[:, k, m * P : (m + 1) * P],
                                    rhs=xt_sb[:, k, ssl],
                                    start=(k == 0),
                                    stop=(k == KC - 1),
                                )
                        nc.scalar.activation(
                            out=act_sb[:, m, glp * 2 : (glp + 1) * 2, :],
                            in_=ps[:, :, :],
                            func=Tanh,
                            bias=bc_sb[:, m : m + 1],
                        )

                        # ---- interleave slots ----
                        if not last:
                            if glp == 0:
                                if m == 2:
                                    if i >= 1:
                                        emit_scores(i - 1, 0)
                                        emit_scores(i - 1, 1)
                                    if h == 0 and b + 1 < BPC:
                                        load_xt(b + 1)
                                elif m == 3 and i >= 1:
                                    emit_scores(i - 1, 2)
                                    emit_scores(i - 1, 3)
                            else:
                                if m == 0 and i >= 1:
                                    emit_pool(i - 1)
                                    wb_tiles.pop(i - 1)
                                    act_tiles.pop(i - 1)
                                    if halves[i - 1][1] == NXT - 1:
                                        emit_out(halves[i - 1][0])
                                emit_ts(i, m)
                                if m == 1:
                                    emit_sadd(i, 0)
                                elif m == 3:
                                    emit_sadd(i, 1)
                        else:
                            # final half: per-glp score pipeline to keep
                            # the drain tail short
                            if glp == 0:
                                if m == 2:
                                    emit_scores(i - 1, 0)
                                    emit_scores(i - 1, 1)
                                elif m == 3:
                                    emit_scores(i - 1, 2)
                                    emit_scores(i - 1, 3)
                    if last:
                        if glp == 0:
                            # previous half's pooling before the drain chain
                            emit_pool(i - 1)
                            wb_tiles.pop(i - 1)
                            act_tiles.pop(i - 1)
                        # ts over this glp's columns only
                        c0, c1 = glp * 2 * SG, (glp * 2 + 2) * SG
                        env.setdefault("lts", {})
                        for m in range(MC):
                            t = tsp.tile(
                                [P, 2 * SG], BF16, tag=f"lts{m}", name=f"lts{m}_t"
                            )
                            env["lts"][m] = t
                            nc.vector.tensor_scalar(
                                out=t[:, :],
                                in0=act_sb[:, m, glp * 2 : (glp + 1) * 2, :],
                                scalar1=bc_sb[:, MC + m : MC + m + 1],
                                scalar2=None,
                                op0=Mult,
                            )
                        s01 = saddp.tile([P, 2 * SG], BF16, tag="lsadd0")
                        s23 = saddp.tile([P, 2 * SG], BF16, tag="lsadd1")
                        nc.vector.tensor_tensor(
                            out=s01[:, :], in0=env["lts"][0][:, :],
                            in1=env["lts"][1][:, :], op=Add,
                        )
                        nc.vector.tensor_tensor(
                            out=s23[:, :], in0=env["lts"][2][:, :],
                            in1=env["lts"][3][:, :], op=Add,
                        )
                        sadd_tiles[i] = {0: s01, 1: s23}
                        # scores + wb for this glp's two subgroups; wb tile
                        # covers the full half, written per glp
                        for j in range(2):
                            gl = glp * 2 + j
                            scp = scps.tile([P, SG], F32, tag="scp", name="scp_t")
                            csl = slice(j * SG, (j + 1) * SG)
                            nc.tensor.matmul(
                                scp[:, :], lhsT=env["ones"][:, :], rhs=s01[:, csl],
                                start=True, stop=False,
                            )
                            nc.tensor.matmul(
                                scp[:, :], lhsT=env["ones"][:, :], rhs=s23[:, csl],
                                start=False, stop=True,
                            )
                            ef = efullp.tile([P, SG], BF16, tag="ef", name="ef_t")
                            nc.scalar.activation(out=ef[:, :], in_=scp[:, :], func=Exp)
                            if glp == 0 and j == 0:
                                wb_tiles[i] = wbcp.tile(
                                    [P, HALF], BF16, tag="wb", name="wb_t"
                                )
                            wb = wb_tiles[i]
                            ssl = slice(h * HALF + gl * SG, h * HALF + (gl + 1) * SG)
                            nc.vector.scalar_tensor_tensor(
                                out=wb[:, gl * SG : (gl + 1) * SG],
                                in0=ef[:, :],
                                scalar=-1.0,
                                in1=mask_tiles[b][:, ssl],
                                op0=Add,
                                op1=Mult,
                                accum_out=den_tiles[b][
                                    :, h * GPH + gl : h * GPH + gl + 1
                                ],
                            )
                        # pool this glp: slot 1 for glp0, slot 2 for glp1
                        emit_pool(i, slot=1 + glp, cols=(glp * 2 * SG, (glp * 2 + 2) * SG))

            emit_out(BPC - 1)

    nc.compile()
    return nc


_NC_CACHE = {}


def _get_nc():
    if "nc" not in _NC_CACHE:
        _NC_CACHE["nc"] = build()
    return _NC_CACHE["nc"]


def kernel(inputs, mask, W, b, context):
    X = np.asarray(inputs, dtype=np.float32)
    mask = np.asarray(mask)
    W = np.asarray(W, dtype=np.float32)
    b = np.asarray(b, dtype=np.float32)
    context = np.asarray(context, dtype=np.float32)

    nc = _get_nc()

    xt_full = np.ascontiguousarray(X.transpose(0, 2, 1)).reshape(B, KC, P, S).astype(BF)
    wt = np.ascontiguousarray(W.T).reshape(KC, P, H).astype(BF)
    bc = np.concatenate(
        [b.reshape(MC, P).T, context.reshape(MC, P).T], axis=1
    ).astype(np.float32)
    bc = np.ascontiguousarray(bc)
    # mask row-replicated across 128 partitions
    mask_rep = np.ascontiguousarray(
        np.broadcast_to(mask.astype(np.float32)[:, None, :], (B, P, S))
    ).astype(BF)

    in_maps = []
    for c in range(N_CORES):
        in_maps.append(
            {
                "xt": xt_full[c * BPC : (c + 1) * BPC],
                "wt": wt,
                "bc": bc,
                "mask": mask_rep[c * BPC : (c + 1) * BPC],
            }
        )

    res = run_bass_kernel_spmd(
        nc, in_maps, core_ids=list(range(N_CORES)), trace=TRACE, tmpdir=TRACE_DIR
    )
    LAST["exec_time_ns"] = res.exec_time_ns
    LAST["result"] = res

    # host-side correction for the w = (exp(s)-1)*mask + 1 rewrite:
    # num += sum_s X[b,s,:], den += S
    xsum = X.astype(BF).astype(np.float32).sum(axis=1)  # [B, H]

    out = np.empty((B, H), np.float32)
    for c in range(N_CORES):
        num = res.results[c]["num"].sum(axis=3)  # [BPC, 128, KC]
        den = res.results[c]["den"].sum(axis=1) + float(S)  # [BPC]
        # num[b, p, k] -> out[b, k*128+p]
        numf = num.transpose(0, 2, 1).reshape(BPC, H) + xsum[c * BPC : (c + 1) * BPC]
        out[c * BPC : (c + 1) * BPC] = numf / den[:, None]
    return out


# revision 13
# speedup vs baseline: 1.4752x; 1.4752x over previous
"""Trainium2 Bass kernel for nn_AttnNet: attention-pooling over sequence (v8).

Reference computation (per batch b):
    act    = tanh(X @ W.T + b)          # [S, H]
    scores = act @ context              # [S]
    w      = exp(scores * mask)         # masked_fill(-1e-32) == *mask (exp(0)=1)
    out    = (X.T @ w) / sum(w)         # [H]

Sharding: pure data-parallel, 4 batches per core across 8 cores.

v8 vs v6 (175998 ns):
  * scores hybrid: channel chunks m=0,1 are ctx-scaled on the DVE
    (tensor_scalar hits the 4x perf mode: 676ns/[128,2048]) and
    pair-added; chunks m=2,3 keep the v6 replicated-ctx matmul form.
    Score cost per 512-col subgroup drops from 4 PE matmuls to 3
    (one ones-broadcast MM over the m01 partial + two ctxr MMs),
    PE work 136us -> ~130us, paid with ~2.1us/half of idle DVE.
  * act GEMM emits (glp, m) blocks of two 512-col subgroups into a
    2-bank PSUM tile so one activation instruction tanh's 1024 columns
    (Act engine ~110us -> ~93us; same per-partition bias chunk).
  * GpSimd does bulk NOTHING: its tensor ops run at half DVE speed and
    stall concurrent DVE ops on the shared SBUF ports (measured 6x).
  * xt uploads stay per-k contiguous (8KB partition rows -> big DMA
    packets); batch 0 is segmented [1024,1024,2048] cols so the first
    MM block's tile lands at ~4us instead of 12us (deps are per-tile).
  * drain tail: the last half computes scores per gl-PAIR so its first
    half pools while the second still matmuls.

Device layout (per core):
    xt   [BPC, KC, 128, S]  bf16  xt[b,k,p,s] = X[b, s, 128k+p]  (X^T)
    wt   [KC, 128, H]       bf16  wt[k,p,o]   = W[o, 128k+p]     (W^T)
    bc   [128, 2*MC] f32    bias (cols 0:MC) and context (cols MC:2MC),
                            bc[p, MC+m] = context[128m+p]
    ctxr [128, 2*128] bf16  ctxr[p, m*128+j] = context[128(m+2)+p]
                            (column-replicated ctx for m=2,3 score MMs)
    mask [BPC, 128, S] bf16 (row-replicated across partitions)
outputs:
    num  [BPC, 128, KC, NSLOT] f32  partial pooled sums (host combines)
    den  [BPC, NXT*GPH]        f32  partial denominators (host combines)
"""

import numpy as np
import ml_dtypes

import concourse.bass as bass
import concourse.tile as tile
from concourse import bacc, mybir
from concourse.bass_utils import run_bass_kernel_spmd

N_CORES = 8
B, S, H = 32, 4096, 512
BPC = B // N_CORES
P = 128
KC = H // P          # 4 contraction chunks
MC = H // P          # 4 output-channel chunks
SG = 512             # one PSUM bank of f32 columns
NXT = 2
HALF = S // NXT      # 2048
GPH = HALF // SG     # 4 subgroups per half
NSLOT = 3            # num accum slots: h0, h1/glp0, glp1

F32 = mybir.dt.float32
BF16 = mybir.dt.bfloat16
BF = ml_dtypes.bfloat16

TRACE = False
TRACE_DIR = None
LAST = {}


def build():
    nc = bacc.Bacc("TRN2", target_bir_lowering=False, num_devices=N_CORES)
    xt_d = nc.declare_dram_parameter("xt", [BPC, KC, P, S], BF16, isOutput=False)
    wt_d = nc.declare_dram_parameter("wt", [KC, P, H], BF16, isOutput=False)
    bc_d = nc.declare_dram_parameter("bc", [P, 2 * MC], F32, isOutput=False)
    ctxr_d = nc.declare_dram_parameter("ctxr", [P, 2 * P], BF16, isOutput=False)
    mask_d = nc.declare_dram_parameter("mask", [BPC, P, S], BF16, isOutput=False)
    num_d = nc.declare_dram_parameter("num", [BPC, P, KC, NSLOT], F32, isOutput=True)
    den_d = nc.declare_dram_parameter("den", [BPC, NXT * GPH], F32, isOutput=True)

    Tanh = mybir.ActivationFunctionType.Tanh
    Exp = mybir.ActivationFunctionType.Exp
    Mult = mybir.AluOpType.mult
    Add = mybir.AluOpType.add

    with tile.TileContext(nc) as tc:
        with (
            tc.tile_pool(name="singles", bufs=1) as singles,
            tc.tile_pool(name="xtp", bufs=2) as xtp,
            tc.tile_pool(name="xtp0", bufs=1) as xtp0,
            tc.tile_pool(name="actpool", bufs=2) as actpool,
            tc.tile_pool(name="maskpool", bufs=2) as maskpool,
            tc.tile_pool(name="tsp", bufs=2) as tsp,
            tc.tile_pool(name="saddp", bufs=2) as saddp,
            tc.tile_pool(name="efull", bufs=2) as efullp,
            tc.tile_pool(name="wbc", bufs=2) as wbcp,
            tc.tile_pool(name="trash", bufs=2) as trashp,
            tc.tile_pool(name="nums", bufs=2) as nums,
            tc.tile_pool(name="dens", bufs=2) as dens,
            tc.tile_pool(name="actps", bufs=3, space="PSUM") as actps,
            tc.tile_pool(name="scps", bufs=2, space="PSUM") as scps,
        ):
            halves = [(b, h) for b in range(BPC) for h in range(NXT)]
            NH = len(halves)

            # xt per batch: list of (tile, col_start, col_end) segments
            xt_segs = {}
            mask_tiles = {}
            num_tiles = {}
            den_tiles = {}
            act_tiles = {}    # per half
            sadd_tiles = {}   # per half: s01 partial (m0+m1, ctx-scaled)
            wb_tiles = {}     # per half
            env = {}

            def load_xt(b):
                if b == 0:
                    segs = []
                    for c0, c1 in ((0, 1024), (1024, 2048), (2048, 4096)):
                        t = xtp0.tile(
                            [P, KC, c1 - c0], BF16, tag=f"xt0_{c0}", name="xt0_sb"
                        )
                        segs.append((t, c0, c1))
                    xt_segs[0] = segs
                    # interleave issue order: first segment's k's, then wt/bc
                    # are issued by caller between these
                    for k in range(KC):
                        nc.sync.dma_start(
                            out=segs[0][0][:, k, :], in_=xt_d.ap()[0, k, :, 0:1024]
                        )
                else:
                    t = xtp.tile([P, KC, S], BF16, tag="xt", name="xt_sb")
                    xt_segs[b] = [(t, 0, S)]
                    for k in range(KC):
                        nc.sync.dma_start(out=t[:, k, :], in_=xt_d.ap()[b, k])

            def xt_ap(b, k, c0, c1):
                """AP for xt[b, k, c0:c1] — always within one segment."""
                for t, s0, s1 in xt_segs[b]:
                    if c0 >= s0 and c1 <= s1:
                        return t[:, k, c0 - s0 : c1 - s0]
                raise AssertionError((b, k, c0, c1))

            def load_batch_state(b):
                mask_sb = maskpool.tile([P, S], BF16, tag="mask")
                mask_tiles[b] = mask_sb
                nc.sync.dma_start(out=mask_sb[:, :], in_=mask_d.ap()[b])
                num_tiles[b] = nums.tile([P, KC, NSLOT], F32, tag="num", name="num_sb")
                if 0 < b < BPC - 1:
                    # b0 uses slot 2 for its h0 second pooling span; the last
                    # batch uses it for the final-half glp1 pool
                    nc.gpsimd.memset(num_tiles[b][:, :, 2:3], 0.0)
                den_tiles[b] = dens.tile([P, NXT * GPH], F32, tag="den", name="den_sb")

            def emit_ts(i, m, tag, cols=None):
                """t[m] = act[:, m, glp-range, :] * ctx_col[m]  (DVE 4x)"""
                c0, c1 = cols if cols is not None else (0, GPH)
                t = tsp.tile([P, (c1 - c0) * SG], BF16, tag=tag, name=f"{tag}_t")
                nc.vector.tensor_scalar(
                    out=t[:, :],
                    in0=act_tiles[i][:, m, c0:c1, :],
                    scalar1=env["bc_sb"][:, MC + m : MC + m + 1],
                    scalar2=None,
                    op0=Mult,
                )
                return t

            def emit_scores(i, gl):
                """scores for subgroup gl of half i: ones-broadcast MM over
                the m01 partial + two ctxr MMs over act m2/m3 -> exp ->
                masked wb slice + den partial."""
                b, h = halves[i]
                s01 = sadd_tiles[i]
                scp = scps.tile([P, SG], F32, tag="scp", name="scp_t")
                csl = slice(gl * SG, (gl + 1) * SG)
                nc.tensor.matmul(
                    scp[:, :], lhsT=env["ones"][:, :], rhs=s01[:, csl],
                    start=True, stop=False,
                )
                nc.tensor.matmul(
                    scp[:, :], lhsT=env["ctxr_sb"][:, 0:P],
                    rhs=act_tiles[i][:, 2, gl, :], start=False, stop=False,
                )
                nc.tensor.matmul(
                    scp[:, :], lhsT=env["ctxr_sb"][:, P : 2 * P],
                    rhs=act_tiles[i][:, 3, gl, :], start=False, stop=True,
                )
                ef = efullp.tile([P, SG], BF16, tag="ef", name="ef_t")
                nc.scalar.activation(out=ef[:, :], in_=scp[:, :], func=Exp)
                if gl == 0:
                    wb_tiles[i] = wbcp.tile([P, HALF], BF16, tag="wb", name="wb_t")
                wb = wb_tiles[i]
                ssl = slice(h * HALF + gl * SG, h * HALF + (gl + 1) * SG)
                nc.vector.scalar_tensor_tensor(
                    out=wb[:, csl],
                    in0=ef[:, :],
                    scalar=-1.0,
                    in1=mask_tiles[b][:, ssl],
                    op0=Add,
                    op1=Mult,
                    accum_out=den_tiles[b][:, h * GPH + gl : h * GPH + gl + 1],
                )

            def emit_pool(i, slot=None, cols=None):
                """pooling for half i: num[:, k, slot] = sum xt[k] * wb
                (4 DVE stt+accum ops, split on xt segment boundaries)."""
                b, h = halves[i]
                wb = wb_tiles[i]
                if slot is None:
                    slot = h
                c0, c1 = cols if cols is not None else (0, HALF)
                # split [c0, c1) on xt segment boundaries (batch 0 h0 only);
                # accum_out overwrites, so each span gets its own slot
                # (span 2 of batch-0 h0 uses the otherwise-memset slot 2)
                edges = sorted(
                    {c0, c1}
                    | {
                        e - h * HALF
                        for t, s0, s1 in xt_segs[b]
                        for e in (s0, s1)
                        if c0 < e - h * HALF < c1
                    }
                )
                spans = list(zip(edges[:-1], edges[1:]))
                assert len(spans) <= 2, spans
                for k in range(KC):
                    for si, (sp0, sp1) in enumerate(spans):
                        kslot = slot if si == 0 else 2
                        trash = trashp.tile([P, HALF], BF16, tag="trash")
                        nc.vector.scalar_tensor_tensor(
                            out=trash[:, 0 : sp1 - sp0],
                            in0=xt_ap(b, k, h * HALF + sp0, h * HALF + sp1),
                            scalar=1.0,
                            in1=wb[:, sp0:sp1],
                            op0=Mult,
                            op1=Mult,
                            accum_out=num_tiles[b][:, k, kslot : kslot + 1],
                        )

            def emit_out(b):
                nc.sync.dma_start(out=num_d.ap()[b], in_=num_tiles.pop(b)[:, :, :])
                nc.sync.dma_start(
                    out=den_d.ap()[b : b + 1, :], in_=den_tiles.pop(b)[0:1, :]
                )

            for i, (b, h) in enumerate(halves):
                last = i == NH - 1
                if i == 0:
                    load_xt(0)
                    wt_sb = singles.tile([P, KC, H], BF16)
                    nc.sync.dma_start(
                        out=wt_sb[:, :, :], in_=wt_d.ap().rearrange("k p h -> p k h")
                    )
                    bc_sb = singles.tile([P, 2 * MC], F32)
                    nc.sync.dma_start(out=bc_sb[:, :], in_=bc_d.ap())
                    env["bc_sb"] = bc_sb
                    ctxr_sb = singles.tile([P, 2 * P], BF16)
                    nc.sync.dma_start(out=ctxr_sb[:, :], in_=ctxr_d.ap())
                    env["ctxr_sb"] = ctxr_sb
                    ones = singles.tile([P, P], BF16)
                    nc.gpsimd.memset(ones[:, :], 1.0)
                    env["ones"] = ones
                    # batch 0 remaining segments
                    segs = xt_segs[0]
                    for k in range(KC):
                        nc.sync.dma_start(
                            out=segs[1][0][:, k, :], in_=xt_d.ap()[0, k, :, 1024:2048]
                        )
                    for k in range(KC):
                        nc.sync.dma_start(
                            out=segs[2][0][:, k, :], in_=xt_d.ap()[0, k, :, 2048:4096]
                        )
                if h == 0:
                    load_batch_state(b)

                act_sb = actpool.tile([P, MC, GPH, SG], BF16, tag="act")
                act_tiles[i] = act_sb

                for glp in range(2):
                    for m in range(MC):
                        ps = actps.tile([P, 2, SG], F32, tag="ps")
                        for j in range(2):
                            gl = glp * 2 + j
                            cc0 = h * HALF + gl * SG
                            for k in range(KC):
                                nc.tensor.matmul(
                                    ps[:, j, :],
                                    lhsT=wt_sb[:, k, m * P : (m + 1) * P],
                                    rhs=xt_ap(b, k, cc0, cc0 + SG),
                                    start=(k == 0),
                                    stop=(k == KC - 1),
                                )
                        nc.scalar.activation(
                            out=act_sb[:, m, glp * 2 : (glp + 1) * 2, :],
                            in_=ps[:, :, :],
                            func=Tanh,
                            bias=bc_sb[:, m : m + 1],
                        )

                        # ---- interleave slots ----
                        if not last:
                            if glp == 0:
                                if m == 0 and i >= 1:
                                    emit_scores(i - 1, 0)
                                    emit_scores(i - 1, 1)
                                elif m == 1 and i >= 1:
                                    emit_scores(i - 1, 2)
                                    emit_scores(i - 1, 3)
                                elif m == 2 and i >= 1:
                                    emit_pool(i - 1)
                                    wb_tiles.pop(i - 1)
                                    act_tiles.pop(i - 1)
                                    if halves[i - 1][1] == NXT - 1:
                                        emit_out(halves[i - 1][0])
                                elif m == 3:
                                    if h == 0 and b + 1 < BPC:
                                        load_xt(b + 1)
                            else:
                                if m == 0:
                                    env["t0"] = emit_ts(i, 0, "ts0")
                                elif m == 1:
                                    t1 = emit_ts(i, 1, "ts1")
                                    s01 = saddp.tile(
                                        [P, HALF], BF16, tag="sadd", name="s01_t"
                                    )
                                    sadd_tiles[i] = s01
                                    nc.vector.tensor_tensor(
                                        out=s01[:, :], in0=env["t0"][:, :],
                                        in1=t1[:, :], op=Add,
                                    )
                        else:
                            if glp == 0:
                                if m == 0:
                                    emit_scores(i - 1, 0)
                                    emit_scores(i - 1, 1)
                                elif m == 1:
                                    emit_scores(i - 1, 2)
                                    emit_scores(i - 1, 3)
                                elif m == 2:
                                    emit_pool(i - 1)
                                    wb_tiles.pop(i - 1)
                                    act_tiles.pop(i - 1)

                    if last:
                        # final half: per-glp score pipeline -> short drain
                        lt0 = emit_ts(i, 0, "lts0", cols=(glp * 2, glp * 2 + 2))
                        lt1 = emit_ts(i, 1, "lts1", cols=(glp * 2, glp * 2 + 2))
                        s01 = saddp.tile([P, 2 * SG], BF16, tag="lsadd")
                        nc.vector.tensor_tensor(
                            out=s01[:, :], in0=lt0[:, :], in1=lt1[:, :], op=Add
                        )
                        for j in range(2):
                            gl = glp * 2 + j
                            scp = scps.tile([P, SG], F32, tag="scp", name="scp_t")
                            csl = slice(j * SG, (j + 1) * SG)
                            nc.tensor.matmul(
                                scp[:, :], lhsT=env["ones"][:, :], rhs=s01[:, csl],
                                start=True, stop=False,
                            )
                            nc.tensor.matmul(
                                scp[:, :], lhsT=env["ctxr_sb"][:, 0:P],
                                rhs=act_sb[:, 2, gl, :], start=False, stop=False,
                            )
                            nc.tensor.matmul(
                                scp[:, :], lhsT=env["ctxr_sb"][:, P : 2 * P],
                                rhs=act_sb[:, 3, gl, :], start=False, stop=True,
                            )
                            ef = efullp.tile([P, SG], BF16, tag="ef", name="ef_t")
                            nc.scalar.activation(out=ef[:, :], in_=scp[:, :], func=Exp)
                            if glp == 0 and j == 0:
                                wb_tiles[i] = wbcp.tile(
                                    [P, HALF], BF16, tag="wb", name="wb_t"
                                )
                            wb = wb_tiles[i]
                            ssl = slice(h * HALF + gl * SG, h * HALF + (gl + 1) * SG)
                            nc.vector.scalar_tensor_tensor(
                                out=wb[:, gl * SG : (gl + 1) * SG],
                                in0=ef[:, :],
                                scalar=-1.0,
                                in1=mask_tiles[b][:, ssl],
                                op0=Add,
                                op1=Mult,
                                accum_out=den_tiles[b][
                                    :, h * GPH + gl : h * GPH + gl + 1
                                ],
                            )
                        # pool this glp: slot 1 for glp0, slot 2 for glp1
                        emit_pool(
                            i, slot=1 + glp, cols=(glp * 2 * SG, (glp * 2 + 2) * SG)
                        )

            emit_out(BPC - 1)

    nc.compile()
    return nc


_NC_CACHE = {}


def _get_nc():
    if "nc" not in _NC_CACHE:
        _NC_CACHE["nc"] = build()
    return _NC_CACHE["nc"]


def kernel(inputs, mask, W, b, context):
    X = np.asarray(inputs, dtype=np.float32)
    mask = np.asarray(mask)
    W = np.asarray(W, dtype=np.float32)
    b = np.asarray(b, dtype=np.float32)
    context = np.asarray(context, dtype=np.float32)

    nc = _get_nc()

    xt_full = np.ascontiguousarray(X.transpose(0, 2, 1)).reshape(B, KC, P, S).astype(BF)
    wt = np.ascontiguousarray(W.T).reshape(KC, P, H).astype(BF)
    bc = np.concatenate(
        [b.reshape(MC, P).T, context.reshape(MC, P).T], axis=1
    ).astype(np.float32)
    bc = np.ascontiguousarray(bc)
    # ctxr[p, m*128+j] = context[128*(m+2)+p], replicated over j
    cr = context.reshape(MC, P)
    ctxr = np.ascontiguousarray(
        np.broadcast_to(
            cr[2:4, :, None], (2, P, P)
        ).transpose(1, 0, 2).reshape(P, 2 * P)
    ).astype(BF)
    # mask row-replicated across 128 partitions
    mask_rep = np.ascontiguousarray(
        np.broadcast_to(mask.astype(np.float32)[:, None, :], (B, P, S))
    ).astype(BF)

    in_maps = []
    for c in range(N_CORES):
        in_maps.append(
            {
                "xt": xt_full[c * BPC : (c + 1) * BPC],
                "wt": wt,
                "bc": bc,
                "ctxr": ctxr,
                "mask": mask_rep[c * BPC : (c + 1) * BPC],
            }
        )

    res = run_bass_kernel_spmd(
        nc, in_maps, core_ids=list(range(N_CORES)), trace=TRACE, tmpdir=TRACE_DIR
    )
    LAST["exec_time_ns"] = res.exec_time_ns
    LAST["result"] = res

    # host-side correction for the w = (exp(s)-1)*mask + 1 rewrite:
    # num += sum_s X[b,s,:], den += S
    xsum = X.astype(BF).astype(np.float32).sum(axis=1)  # [B, H]

    out = np.empty((B, H), np.float32)
    for c in range(N_CORES):
        num = res.results[c]["num"].sum(axis=3)  # [BPC, 128, KC]
        den = res.results[c]["den"].sum(axis=1) + float(S)  # [BPC]
        # num[b, p, k] -> out[b, k*128+p]
        numf = num.transpose(0, 2, 1).reshape(BPC, H) + xsum[c * BPC : (c + 1) * BPC]
        out[c * BPC : (c + 1) * BPC] = numf / den[:, None]
    return out


# revision 23
# speedup vs baseline: 1.5103x; 1.0238x over previous
"""Trainium2 Bass kernel for nn_AttnNet: attention-pooling over sequence (v8).

Reference computation (per batch b):
    act    = tanh(X @ W.T + b)          # [S, H]
    scores = act @ context              # [S]
    w      = exp(scores * mask)         # masked_fill(-1e-32) == *mask (exp(0)=1)
    out    = (X.T @ w) / sum(w)         # [H]

Sharding: pure data-parallel, 4 batches per core across 8 cores.

v8 vs v6 (175998 ns):
  * scores hybrid: channel chunks m=0,1 are ctx-scaled on the DVE
    (tensor_scalar hits the 4x perf mode: 676ns/[128,2048]) and
    pair-added; chunks m=2,3 keep the v6 replicated-ctx matmul form.
    Score cost per 512-col subgroup drops from 4 PE matmuls to 3
    (one ones-broadcast MM over the m01 partial + two ctxr MMs),
    PE work 136us -> ~130us, paid with ~2.1us/half of idle DVE.
  * act GEMM emits (glp, m) blocks of two 512-col subgroups into a
    2-bank PSUM tile so one activation instruction tanh's 1024 columns
    (Act engine ~110us -> ~93us; same per-partition bias chunk).
  * GpSimd does bulk NOTHING: its tensor ops run at half DVE speed and
    stall concurrent DVE ops on the shared SBUF ports (measured 6x).
  * xt uploads stay per-k contiguous (8KB partition rows -> big DMA
    packets); batch 0 is segmented [1024,1024,2048] cols so the first
    MM block's tile lands at ~4us instead of 12us (deps are per-tile).
  * drain tail: the last half computes scores per gl-PAIR so its first
    half pools while the second still matmuls.

Device layout (per core):
    xt   [BPC, KC, 128, S]  bf16  xt[b,k,p,s] = X[b, s, 128k+p]  (X^T)
    wt   [KC, 128, H]       bf16  wt[k,p,o]   = W[o, 128k+p]     (W^T)
    bc   [128, 2*MC] f32    bias (cols 0:MC) and context (cols MC:2MC),
                            bc[p, MC+m] = context[128m+p]
    ctxr [128, 2*128] bf16  ctxr[p, m*128+j] = context[128(m+2)+p]
                            (column-replicated ctx for m=2,3 score MMs)
    mask [BPC, 128, S] bf16 (row-replicated across partitions)
outputs:
    num  [BPC, 128, KC, NSLOT] f32  partial pooled sums (host combines)
    den  [BPC, NXT*GPH]        f32  partial denominators (host combines)
"""

import numpy as np
import ml_dtypes

import concourse.bass as bass
import concourse.tile as tile
from concourse import bacc, mybir
from concourse.bass_utils import run_bass_kernel_spmd

N_CORES = 8
B, S, H = 32, 4096, 512
BPC = B // N_CORES
P = 128
KC = H // P          # 4 contraction chunks
MC = H // P          # 4 output-channel chunks
SG = 512             # one PSUM bank of f32 columns
NXT = 2
HALF = S // NXT      # 2048
GPH = HALF // SG     # 4 subgroups per half
NSLOT = 3            # num accum slots: h0, h1/glp0, glp1

F32 = mybir.dt.float32
BF16 = mybir.dt.bfloat16
BF = ml_dtypes.bfloat16

TRACE = False
TRACE_DIR = None
LAST = {}


def build():
    nc = bacc.Bacc("TRN2", target_bir_lowering=False, num_devices=N_CORES)
    xt_d = nc.declare_dram_parameter("xt", [BPC, KC, P, S], BF16, isOutput=False)
    # host-prearranged [p, k, h] so the upload is one contiguous fast DMA
    wt_d = nc.declare_dram_parameter("wt", [P, KC, H], BF16, isOutput=False)
    bc_d = nc.declare_dram_parameter("bc", [P, 2 * MC], F32, isOutput=False)
    ctxr_d = nc.declare_dram_parameter("ctxr", [P, 2 * P], BF16, isOutput=False)
    mask_d = nc.declare_dram_parameter("mask", [BPC, P, S], BF16, isOutput=False)
    num_d = nc.declare_dram_parameter("num", [BPC, P, KC, NSLOT], F32, isOutput=True)
    den_d = nc.declare_dram_parameter("den", [BPC, NXT * GPH], F32, isOutput=True)

    Tanh = mybir.ActivationFunctionType.Tanh
    Exp = mybir.ActivationFunctionType.Exp
    Copy = mybir.ActivationFunctionType.Copy
    Mult = mybir.AluOpType.mult
    Add = mybir.AluOpType.add

    with tile.TileContext(nc) as tc:
        with (
            tc.tile_pool(name="singles", bufs=1) as singles,
            tc.tile_pool(name="xtp", bufs=2) as xtp,
            tc.tile_pool(name="xtp0", bufs=1) as xtp0,
            tc.tile_pool(name="actpool", bufs=2) as actpool,
            tc.tile_pool(name="maskpool", bufs=2) as maskpool,
            tc.tile_pool(name="tsp", bufs=1) as tsp,
            tc.tile_pool(name="saddp", bufs=2) as saddp,
            tc.tile_pool(name="efull", bufs=2) as efullp,
            tc.tile_pool(name="wbc", bufs=2) as wbcp,
            tc.tile_pool(name="trash", bufs=1) as trashp,
            tc.tile_pool(name="nums", bufs=2) as nums,
            tc.tile_pool(name="dens", bufs=2) as dens,
            tc.tile_pool(name="actps", bufs=3, space="PSUM") as actps,
            tc.tile_pool(name="scps", bufs=2, space="PSUM") as scps,
        ):
            halves = [(b, h) for b in range(BPC) for h in range(NXT)]
            NH = len(halves)

            # xt per batch: list of (tile, col_start, col_end) segments
            xt_segs = {}
            mask_tiles = {}
            num_tiles = {}
            den_tiles = {}
            act_tiles = {}    # per half
            sadd_tiles = {}   # per half: s01 partial (m0+m1, ctx-scaled)
            wb_tiles = {}     # per half
            env = {}

            def load_xt(b):
                if b == 0:
                    segs = []
                    for c0, c1 in ((0, 1024), (1024, 2048), (2048, 4096)):
                        t = xtp0.tile(
                            [P, KC, c1 - c0], BF16, tag=f"xt0_{c0}", name="xt0_sb"
                        )
                        segs.append((t, c0, c1))
                    xt_segs[0] = segs
                    # interleave issue order: first segment's k's, then wt/bc
                    # are issued by caller between these
                    for k in range(KC):
                        nc.sync.dma_start(
                            out=segs[0][0][:, k, :], in_=xt_d.ap()[0, k, :, 0:1024]
                        )
                else:
                    t = xtp.tile([P, KC, S], BF16, tag="xt", name="xt_sb")
                    xt_segs[b] = [(t, 0, S)]
                    for k in range(KC):
                        nc.sync.dma_start(out=t[:, k, :], in_=xt_d.ap()[b, k])

            def xt_ap(b, k, c0, c1):
                """AP for xt[b, k, c0:c1] — always within one segment."""
                for t, s0, s1 in xt_segs[b]:
                    if c0 >= s0 and c1 <= s1:
                        return t[:, k, c0 - s0 : c1 - s0]
                raise AssertionError((b, k, c0, c1))

            def load_batch_state(b):
                mask_sb = maskpool.tile([P, S], BF16, tag="mask")
                mask_tiles[b] = mask_sb
                nc.sync.dma_start(out=mask_sb[:, :], in_=mask_d.ap()[b])
                num_tiles[b] = nums.tile([P, KC, NSLOT], F32, tag="num", name="num_sb")
                if 0 < b < BPC - 1:
                    # b0 uses slot 2 for its h0 second pooling span; the last
                    # batch uses it for the final-half glp1 pool
                    nc.gpsimd.memset(num_tiles[b][:, :, 2:3], 0.0)
                den_tiles[b] = dens.tile([P, NXT * GPH], F32, tag="den", name="den_sb")

            def emit_ts(i, m, tag, cols=None):
                """t[m] = act[:, m, glp-range, :] * ctx_col[m]  (DVE 4x)"""
                c0, c1 = cols if cols is not None else (0, GPH)
                t = tsp.tile([P, (c1 - c0) * SG], BF16, tag=tag, name=f"{tag}_t")
                nc.vector.tensor_scalar(
                    out=t[:, :],
                    in0=act_tiles[i][:, m, c0:c1, :],
                    scalar1=env["bc_sb"][:, MC + m : MC + m + 1],
                    scalar2=None,
                    op0=Mult,
                )
                return t

            def emit_scores(i, gl):
                """scores for subgroup gl of half i: ones-broadcast MM over
                the m01 partial + two ctxr MMs over act m2/m3 -> exp ->
                masked wb slice + den partial."""
                b, h = halves[i]
                s01 = sadd_tiles[i]
                scp = scps.tile([P, SG], F32, tag="scp", name="scp_t")
                csl = slice(gl * SG, (gl + 1) * SG)
                nc.tensor.matmul(
                    scp[:, :], lhsT=env["ones"][:, :], rhs=s01[:, csl],
                    start=True, stop=False,
                )
                nc.tensor.matmul(
                    scp[:, :], lhsT=env["ctxr_sb"][:, 0:P],
                    rhs=act_tiles[i][:, 2, gl, :], start=False, stop=False,
                )
                nc.tensor.matmul(
                    scp[:, :], lhsT=env["ctxr_sb"][:, P : 2 * P],
                    rhs=act_tiles[i][:, 3, gl, :], start=False, stop=True,
                )
                ef = efullp.tile([P, SG], BF16, tag="ef", name="ef_t")
                nc.scalar.activation(out=ef[:, :], in_=scp[:, :], func=Exp)
                if gl == 0:
                    wb_tiles[i] = wbcp.tile([P, HALF], BF16, tag="wb", name="wb_t")
                wb = wb_tiles[i]
                ssl = slice(h * HALF + gl * SG, h * HALF + (gl + 1) * SG)
                nc.vector.scalar_tensor_tensor(
                    out=wb[:, csl],
                    in0=ef[:, :],
                    scalar=-1.0,
                    in1=mask_tiles[b][:, ssl],
                    op0=Add,
                    op1=Mult,
                    accum_out=den_tiles[b][:, h * GPH + gl : h * GPH + gl + 1],
                )

            def emit_pool(i, slot=None, cols=None, split=False):
                """pooling for half i: num[:, k, slot] = sum xt[k] * wb
                (4 DVE stt+accum ops, split on xt segment boundaries).
                With split=True, k=2,3 go through a DVE tensor_tensor
                product (2x mode) + Act Copy+accum instead, halving the
                DVE cost in the drain where Act is idle."""
                b, h = halves[i]
                wb = wb_tiles[i]
                if slot is None:
                    slot = h
                c0, c1 = cols if cols is not None else (0, HALF)
                # split [c0, c1) on xt segment boundaries (batch 0 h0 only);
                # accum_out overwrites, so each span gets its own slot
                # (span 2 of batch-0 h0 uses the otherwise-memset slot 2)
                edges = sorted(
                    {c0, c1}
                    | {
                        e - h * HALF
                        for t, s0, s1 in xt_segs[b]
                        for e in (s0, s1)
                        if c0 < e - h * HALF < c1
                    }
                )
                spans = list(zip(edges[:-1], edges[1:]))
                assert len(spans) <= 2, spans
                for k in range(KC):
                    for si, (sp0, sp1) in enumerate(spans):
                        kslot = slot if si == 0 else 2
                        if split and k >= 2:
                            prod = trashp.tile(
                                [P, HALF], BF16, tag=f"prod{k % 2}", name="prod_t"
                            )
                            nc.vector.tensor_tensor(
                                out=prod[:, 0 : sp1 - sp0],
                                in0=xt_ap(b, k, h * HALF + sp0, h * HALF + sp1),
                                in1=wb[:, sp0:sp1],
                                op=Mult,
                            )
                            trash = trashp.tile([P, HALF], BF16, tag="trashact")
                            nc.scalar.activation(
                                out=trash[:, 0 : sp1 - sp0],
                                in_=prod[:, 0 : sp1 - sp0],
                                func=Copy,
                                accum_out=num_tiles[b][:, k, kslot : kslot + 1],
                            )
                        else:
                            trash = trashp.tile([P, HALF], BF16, tag="trash")
                            nc.vector.scalar_tensor_tensor(
                                out=trash[:, 0 : sp1 - sp0],
                                in0=xt_ap(b, k, h * HALF + sp0, h * HALF + sp1),
                                scalar=1.0,
                                in1=wb[:, sp0:sp1],
                                op0=Mult,
                                op1=Mult,
                                accum_out=num_tiles[b][:, k, kslot : kslot + 1],
                            )

            def emit_out(b):
                nc.sync.dma_start(out=num_d.ap()[b], in_=num_tiles.pop(b)[:, :, :])
                nc.sync.dma_start(
                    out=den_d.ap()[b : b + 1, :], in_=den_tiles.pop(b)[0:1, :]
                )

            for i, (b, h) in enumerate(halves):
                last = i == NH - 1
                if i == 0:
                    wt_sb = singles.tile([P, KC, H], BF16)
                    nc.sync.dma_start(out=wt_sb[:, :, :], in_=wt_d.ap())
                    bc_sb = singles.tile([P, 2 * MC], F32)
                    nc.sync.dma_start(out=bc_sb[:, :], in_=bc_d.ap())
                    env["bc_sb"] = bc_sb
                    ctxr_sb = singles.tile([P, 2 * P], BF16)
                    nc.sync.dma_start(out=ctxr_sb[:, :], in_=ctxr_d.ap())
                    env["ctxr_sb"] = ctxr_sb
                    load_xt(0)
                    ones = singles.tile([P, P], BF16)
                    nc.gpsimd.memset(ones[:, :], 1.0)
                    env["ones"] = ones
                    # batch 0 remaining segments
                    segs = xt_segs[0]
                    for k in range(KC):
                        nc.sync.dma_start(
                            out=segs[1][0][:, k, :], in_=xt_d.ap()[0, k, :, 1024:2048]
                        )
                    for k in range(KC):
                        nc.sync.dma_start(
                            out=segs[2][0][:, k, :], in_=xt_d.ap()[0, k, :, 2048:4096]
                        )
                if h == 0:
                    load_batch_state(b)

                act_sb = actpool.tile([P, MC, GPH, SG], BF16, tag="act")
                act_tiles[i] = act_sb

                for glp in range(2):
                    for m in range(MC):
                        ps = actps.tile([P, 2, SG], F32, tag="ps")
                        for j in range(2):
                            gl = glp * 2 + j
                            cc0 = h * HALF + gl * SG
                            for k in range(KC):
                                nc.tensor.matmul(
                                    ps[:, j, :],
                                    lhsT=wt_sb[:, k, m * P : (m + 1) * P],
                                    rhs=xt_ap(b, k, cc0, cc0 + SG),
                                    start=(k == 0),
                                    stop=(k == KC - 1),
                                )
                        nc.scalar.activation(
                            out=act_sb[:, m, glp * 2 : (glp + 1) * 2, :],
                            in_=ps[:, :, :],
                            func=Tanh,
                            bias=bc_sb[:, m : m + 1],
                        )

                        # ---- interleave slots ----
                        if not last:
                            if glp == 0:
                                if m == 0 and i >= 1:
                                    emit_scores(i - 1, 0)
                                    emit_scores(i - 1, 1)
                                elif m == 1 and i >= 1:
                                    emit_scores(i - 1, 2)
                                    emit_scores(i - 1, 3)
                                elif m == 2 and i >= 1:
                                    emit_pool(i - 1)
                                    wb_tiles.pop(i - 1)
                                    act_tiles.pop(i - 1)
                                    if halves[i - 1][1] == NXT - 1:
                                        emit_out(halves[i - 1][0])
                                elif m == 3:
                                    if h == 0 and b + 1 < BPC:
                                        load_xt(b + 1)
                            else:
                                if m == 0:
                                    env["t0"] = emit_ts(i, 0, "ts0")
                                elif m == 1:
                                    t1 = emit_ts(i, 1, "ts1")
                                    s01 = saddp.tile(
                                        [P, HALF], BF16, tag="sadd", name="s01_t"
                                    )
                                    sadd_tiles[i] = s01
                                    nc.vector.tensor_tensor(
                                        out=s01[:, :], in0=env["t0"][:, :],
                                        in1=t1[:, :], op=Add,
                                    )
                        else:
                            if glp == 0:
                                if m == 0:
                                    emit_scores(i - 1, 0)
                                    emit_scores(i - 1, 1)
                                elif m == 1:
                                    emit_scores(i - 1, 2)
                                    emit_scores(i - 1, 3)

                    if last:
                        # final half: per-glp score pipeline -> short drain
                        lt0 = emit_ts(i, 0, "lts0", cols=(glp * 2, glp * 2 + 2))
                        lt1 = emit_ts(i, 1, "lts1", cols=(glp * 2, glp * 2 + 2))
                        s01 = saddp.tile([P, 2 * SG], BF16, tag="lsadd")
                        nc.vector.tensor_tensor(
                            out=s01[:, :], in0=lt0[:, :], in1=lt1[:, :], op=Add
                        )
                        for j in range(2):
                            gl = glp * 2 + j
                            scp = scps.tile([P, SG], F32, tag="scp", name="scp_t")
                            csl = slice(j * SG, (j + 1) * SG)
                            nc.tensor.matmul(
                                scp[:, :], lhsT=env["ones"][:, :], rhs=s01[:, csl],
                                start=True, stop=False,
                            )
                            nc.tensor.matmul(
                                scp[:, :], lhsT=env["ctxr_sb"][:, 0:P],
                                rhs=act_sb[:, 2, gl, :], start=False, stop=False,
                            )
                            nc.tensor.matmul(
                                scp[:, :], lhsT=env["ctxr_sb"][:, P : 2 * P],
                                rhs=act_sb[:, 3, gl, :], start=False, stop=True,
                            )
                            ef = efullp.tile([P, SG], BF16, tag="ef", name="ef_t")
                            nc.scalar.activation(out=ef[:, :], in_=scp[:, :], func=Exp)
                            if glp == 0 and j == 0:
                                wb_tiles[i] = wbcp.tile(
                                    [P, HALF], BF16, tag="wb", name="wb_t"
                                )
                            wb = wb_tiles[i]
                            ssl = slice(h * HALF + gl * SG, h * HALF + (gl + 1) * SG)
                            nc.vector.scalar_tensor_tensor(
                                out=wb[:, gl * SG : (gl + 1) * SG],
                                in0=ef[:, :],
                                scalar=-1.0,
                                in1=mask_tiles[b][:, ssl],
                                op0=Add,
                                op1=Mult,
                                accum_out=den_tiles[b][
                                    :, h * GPH + gl : h * GPH + gl + 1
                                ],
                            )
                        if glp == 0:
                            # previous half's pooling now that the drain
                            # chain's DVE ops are already queued ahead of it
                            emit_pool(i - 1, split=True)
                            wb_tiles.pop(i - 1)
                            act_tiles.pop(i - 1)
                        # pool this glp: slot 1 for glp0, slot 2 for glp1
                        emit_pool(
                            i,
                            slot=1 + glp,
                            cols=(glp * 2 * SG, (glp * 2 + 2) * SG),
                            split=True,
                        )

            emit_out(BPC - 1)

    nc.compile()
    return nc


_NC_CACHE = {}


def _get_nc():
    if "nc" not in _NC_CACHE:
        _NC_CACHE["nc"] = build()
    return _NC_CACHE["nc"]


def kernel(inputs, mask, W, b, context):
    X = np.asarray(inputs, dtype=np.float32)
    mask = np.asarray(mask)
    W = np.asarray(W, dtype=np.float32)
    b = np.asarray(b, dtype=np.float32)
    context = np.asarray(context, dtype=np.float32)

    nc = _get_nc()

    xt_full = np.ascontiguousarray(X.transpose(0, 2, 1)).reshape(B, KC, P, S).astype(BF)
    # wt[p, k, o] = W[o, 128k+p]
    wt = np.ascontiguousarray(W.T.reshape(KC, P, H).transpose(1, 0, 2)).astype(BF)
    bc = np.concatenate(
        [b.reshape(MC, P).T, context.reshape(MC, P).T], axis=1
    ).astype(np.float32)
    bc = np.ascontiguousarray(bc)
    # ctxr[p, m*128+j] = context[128*(m+2)+p], replicated over j
    cr = context.reshape(MC, P)
    ctxr = np.ascontiguousarray(
        np.broadcast_to(
            cr[2:4, :, None], (2, P, P)
        ).transpose(1, 0, 2).reshape(P, 2 * P)
    ).astype(BF)
    # mask row-replicated across 128 partitions
    mask_rep = np.ascontiguousarray(
        np.broadcast_to(mask.astype(np.float32)[:, None, :], (B, P, S))
    ).astype(BF)

    in_maps = []
    for c in range(N_CORES):
        in_maps.append(
            {
                "xt": xt_full[c * BPC : (c + 1) * BPC],
                "wt": wt,
                "bc": bc,
                "ctxr": ctxr,
                "mask": mask_rep[c * BPC : (c + 1) * BPC],
            }
        )

    res = run_bass_kernel_spmd(
        nc, in_maps, core_ids=list(range(N_CORES)), trace=TRACE, tmpdir=TRACE_DIR
    )
    LAST["exec_time_ns"] = res.exec_time_ns
    LAST["result"] = res

    # host-side correction for the w = (exp(s)-1)*mask + 1 rewrite:
    # num += sum_s X[b,s,:], den += S
    xsum = X.astype(BF).astype(np.float32).sum(axis=1)  # [B, H]

    out = np.empty((B, H), np.float32)
    for c in range(N_CORES):
        num = res.results[c]["num"].sum(axis=3)  # [BPC, 128, KC]
        den = res.results[c]["den"].sum(axis=1) + float(S)  # [BPC]
        # num[b, p, k] -> out[b, k*128+p]
        numf = num.transpose(0, 2, 1).reshape(BPC, H) + xsum[c * BPC : (c + 1) * BPC]
        out[c * BPC : (c + 1) * BPC] = numf / den[:, None]
    return out


# revision 34
# speedup vs baseline: 1.5264x; 1.0106x over previous
"""Trainium2 Bass kernel for nn_AttnNet: attention-pooling over sequence (v8).

Reference computation (per batch b):
    act    = tanh(X @ W.T + b)          # [S, H]
    scores = act @ context              # [S]
    w      = exp(scores * mask)         # masked_fill(-1e-32) == *mask (exp(0)=1)
    out    = (X.T @ w) / sum(w)         # [H]

Sharding: pure data-parallel, 4 batches per core across 8 cores.

v8 vs v6 (175998 ns):
  * scores hybrid: channel chunks m=0,1 are ctx-scaled on the DVE
    (tensor_scalar hits the 4x perf mode: 676ns/[128,2048]) and
    pair-added; chunks m=2,3 keep the v6 replicated-ctx matmul form.
    Score cost per 512-col subgroup drops from 4 PE matmuls to 3
    (one ones-broadcast MM over the m01 partial + two ctxr MMs),
    PE work 136us -> ~130us, paid with ~2.1us/half of idle DVE.
  * act GEMM emits (glp, m) blocks of two 512-col subgroups into a
    2-bank PSUM tile so one activation instruction tanh's 1024 columns
    (Act engine ~110us -> ~93us; same per-partition bias chunk).
  * GpSimd does bulk NOTHING: its tensor ops run at half DVE speed and
    stall concurrent DVE ops on the shared SBUF ports (measured 6x).
  * xt uploads stay per-k contiguous (8KB partition rows -> big DMA
    packets); batch 0 is segmented [1024,1024,2048] cols so the first
    MM block's tile lands at ~4us instead of 12us (deps are per-tile).
  * drain tail: the last half computes scores per gl-PAIR so its first
    half pools while the second still matmuls.

Device layout (per core):
    xt   [BPC, KC, 128, S]  bf16  xt[b,k,p,s] = X[b, s, 128k+p]  (X^T)
    wt   [KC, 128, H]       bf16  wt[k,p,o]   = W[o, 128k+p]     (W^T)
    bc   [128, 2*MC] f32    bias (cols 0:MC) and context (cols MC:2MC),
                            bc[p, MC+m] = context[128m+p]
    ctxr [128, 2*128] bf16  ctxr[p, m*128+j] = context[128(m+2)+p]
                            (column-replicated ctx for m=2,3 score MMs)
    mask [BPC, 128, S] bf16 (row-replicated across partitions)
outputs:
    num  [BPC, 128, KC, NSLOT] f32  partial pooled sums (host combines)
    den  [BPC, NXT*GPH]        f32  partial denominators (host combines)
"""

import numpy as np
import ml_dtypes

import concourse.bass as bass
import concourse.tile as tile
from concourse import bacc, mybir
from concourse.bass_utils import run_bass_kernel_spmd

N_CORES = 8
B, S, H = 32, 4096, 512
BPC = B // N_CORES
P = 128
KC = H // P          # 4 contraction chunks
MC = H // P          # 4 output-channel chunks
SG = 512             # one PSUM bank of f32 columns
NXT = 2
HALF = S // NXT      # 2048
GPH = HALF // SG     # 4 subgroups per half
NSLOT = 4            # num accum slots (pool spans/glps use distinct slots)

F32 = mybir.dt.float32
BF16 = mybir.dt.bfloat16
BF = ml_dtypes.bfloat16

TRACE = False
TRACE_DIR = None
LAST = {}


def build():
    nc = bacc.Bacc("TRN2", target_bir_lowering=False, num_devices=N_CORES)
    xt_d = nc.declare_dram_parameter("xt", [BPC, KC, P, S], BF16, isOutput=False)
    # host-prearranged [p, m, k, 128] (m-major) so uploads are contiguous
    wt_d = nc.declare_dram_parameter("wt", [P, MC, KC, P], BF16, isOutput=False)
    bc_d = nc.declare_dram_parameter("bc", [P, 2 * MC], F32, isOutput=False)
    ctxr_d = nc.declare_dram_parameter("ctxr", [P, 2 * P], BF16, isOutput=False)
    mask_d = nc.declare_dram_parameter("mask", [BPC, P, S], BF16, isOutput=False)
    num_d = nc.declare_dram_parameter("num", [BPC, P, KC, NSLOT], F32, isOutput=True)
    den_d = nc.declare_dram_parameter("den", [BPC, NXT * GPH], F32, isOutput=True)

    Tanh = mybir.ActivationFunctionType.Tanh
    Exp = mybir.ActivationFunctionType.Exp
    Copy = mybir.ActivationFunctionType.Copy
    Mult = mybir.AluOpType.mult
    Add = mybir.AluOpType.add

    with tile.TileContext(nc) as tc:
        with (
            tc.tile_pool(name="singles", bufs=1) as singles,
            tc.tile_pool(name="xtp", bufs=2) as xtp,
            tc.tile_pool(name="xtp0", bufs=1) as xtp0,
            tc.tile_pool(name="actpool", bufs=2) as actpool,
            tc.tile_pool(name="maskpool", bufs=2) as maskpool,
            tc.tile_pool(name="tsp", bufs=1) as tsp,
            tc.tile_pool(name="saddp", bufs=2) as saddp,
            tc.tile_pool(name="efull", bufs=2) as efullp,
            tc.tile_pool(name="wbc", bufs=2) as wbcp,
            tc.tile_pool(name="trash", bufs=1) as trashp,
            tc.tile_pool(name="nums", bufs=2) as nums,
            tc.tile_pool(name="dens", bufs=2) as dens,
            tc.tile_pool(name="actps", bufs=3, space="PSUM") as actps,
            tc.tile_pool(name="scps", bufs=2, space="PSUM") as scps,
        ):
            halves = [(b, h) for b in range(BPC) for h in range(NXT)]
            NH = len(halves)

            # xt per batch: list of (tile, col_start, col_end) segments
            xt_segs = {}
            mask_tiles = {}
            num_tiles = {}
            den_tiles = {}
            act_tiles = {}    # per half
            sadd_tiles = {}   # per half: s01 partial (m0+m1, ctx-scaled)
            wb_tiles = {}     # per half
            env = {}

            def load_xt(b):
                if b == 0:
                    segs = []
                    for c0, c1 in ((0, 512), (512, 1024), (1024, 2048), (2048, 4096)):
                        t = xtp0.tile(
                            [P, KC, c1 - c0], BF16, tag=f"xt0_{c0}", name="xt0_sb"
                        )
                        segs.append((t, c0, c1))
                    xt_segs[0] = segs
                    # first 512 columns as their own tile so the first MM
                    # block's dependency is only 512KB of transfer
                    for k in range(KC):
                        nc.sync.dma_start(
                            out=segs[0][0][:, k, :], in_=xt_d.ap()[0, k, :, 0:512]
                        )
                else:
                    t = xtp.tile([P, KC, S], BF16, tag="xt", name="xt_sb")
                    xt_segs[b] = [(t, 0, S)]
                    for k in range(KC):
                        nc.sync.dma_start(out=t[:, k, :], in_=xt_d.ap()[b, k])

            def xt_ap(b, k, c0, c1):
                """AP for xt[b, k, c0:c1] — always within one segment."""
                for t, s0, s1 in xt_segs[b]:
                    if c0 >= s0 and c1 <= s1:
                        return t[:, k, c0 - s0 : c1 - s0]
                raise AssertionError((b, k, c0, c1))

            def load_batch_state(b):
                mask_sb = maskpool.tile([P, S], BF16, tag="mask")
                mask_tiles[b] = mask_sb
                nc.sync.dma_start(out=mask_sb[:, :], in_=mask_d.ap()[b])
                num_tiles[b] = nums.tile([P, KC, NSLOT], F32, tag="num", name="num_sb")
                if 0 < b < BPC - 1:
                    # b0 h0 pools in 3 spans -> slots 0,2,3; the last batch
                    # uses slot 2 for the final-half glp1 pool
                    nc.gpsimd.memset(num_tiles[b][:, :, 2:4], 0.0)
                elif b == BPC - 1:
                    nc.gpsimd.memset(num_tiles[b][:, :, 3:4], 0.0)
                den_tiles[b] = dens.tile([P, NXT * GPH], F32, tag="den", name="den_sb")

            def emit_ts(i, m, tag, cols=None):
                """t[m] = act[:, m, glp-range, :] * ctx_col[m]  (DVE 4x)"""
                c0, c1 = cols if cols is not None else (0, GPH)
                t = tsp.tile([P, (c1 - c0) * SG], BF16, tag=tag, name=f"{tag}_t")
                nc.vector.tensor_scalar(
                    out=t[:, :],
                    in0=act_tiles[i][:, m, c0:c1, :],
                    scalar1=env["bc_sb"][:, MC + m : MC + m + 1],
                    scalar2=None,
                    op0=Mult,
                )
                return t

            def emit_scores(i, gl):
                """scores for subgroup gl of half i: ones-broadcast MM over
                the m01 partial + two ctxr MMs over act m2/m3 -> exp ->
                masked wb slice + den partial."""
                b, h = halves[i]
                s01 = sadd_tiles[i]
                scp = scps.tile([P, SG], F32, tag="scp", name="scp_t")
                csl = slice(gl * SG, (gl + 1) * SG)
                nc.tensor.matmul(
                    scp[:, :], lhsT=env["ones"][:, :], rhs=s01[:, csl],
                    start=True, stop=False,
                )
                nc.tensor.matmul(
                    scp[:, :], lhsT=env["ctxr_sb"][:, 0:P],
                    rhs=act_tiles[i][:, 2, gl, :], start=False, stop=False,
                )
                nc.tensor.matmul(
                    scp[:, :], lhsT=env["ctxr_sb"][:, P : 2 * P],
                    rhs=act_tiles[i][:, 3, gl, :], start=False, stop=True,
                )
                ef = efullp.tile([P, SG], BF16, tag="ef", name="ef_t")
                nc.scalar.activation(out=ef[:, :], in_=scp[:, :], func=Exp)
                if gl == 0:
                    wb_tiles[i] = wbcp.tile([P, HALF], BF16, tag="wb", name="wb_t")
                wb = wb_tiles[i]
                ssl = slice(h * HALF + gl * SG, h * HALF + (gl + 1) * SG)
                nc.vector.scalar_tensor_tensor(
                    out=wb[:, csl],
                    in0=ef[:, :],
                    scalar=-1.0,
                    in1=mask_tiles[b][:, ssl],
                    op0=Add,
                    op1=Mult,
                    accum_out=den_tiles[b][:, h * GPH + gl : h * GPH + gl + 1],
                )

            def emit_pool(i, slot=None, cols=None, split=False, defer=None):
                """pooling for half i: num[:, k, slot] = sum xt[k] * wb
                (4 DVE stt+accum ops, split on xt segment boundaries).
                With split=True, k=2,3 go through a DVE tensor_tensor
                product (2x mode) + Act Copy+accum instead, halving the
                DVE cost in the drain where Act is idle."""
                b, h = halves[i]
                wb = wb_tiles[i]
                if slot is None:
                    slot = h
                c0, c1 = cols if cols is not None else (0, HALF)
                # split [c0, c1) on xt segment boundaries (batch 0 h0 only);
                # accum_out overwrites, so each span gets its own slot
                # (span 2 of batch-0 h0 uses the otherwise-memset slot 2)
                edges = sorted(
                    {c0, c1}
                    | {
                        e - h * HALF
                        for t, s0, s1 in xt_segs[b]
                        for e in (s0, s1)
                        if c0 < e - h * HALF < c1
                    }
                )
                spans = list(zip(edges[:-1], edges[1:]))
                assert len(spans) <= NSLOT - 1, spans
                for k in range(KC):
                    for si, (sp0, sp1) in enumerate(spans):
                        kslot = slot if si == 0 else (1 + si)
                        if split and k >= 2:
                            prod = trashp.tile(
                                [P, HALF], BF16, tag=f"prod{i % 2}_{k % 2}",
                                name="prod_t",
                            )
                            nc.vector.tensor_tensor(
                                out=prod[:, 0 : sp1 - sp0],
                                in0=xt_ap(b, k, h * HALF + sp0, h * HALF + sp1),
                                in1=wb[:, sp0:sp1],
                                op=Mult,
                            )

                            def _acc(prod=prod, n=sp1 - sp0, b=b, k=k, kslot=kslot):
                                trash = trashp.tile([P, HALF], BF16, tag="trashact")
                                nc.scalar.activation(
                                    out=trash[:, 0:n],
                                    in_=prod[:, 0:n],
                                    func=Copy,
                                    accum_out=num_tiles[b][:, k, kslot : kslot + 1],
                                )

                            if defer is not None:
                                defer.append(_acc)
                            else:
                                _acc()
                        else:
                            trash = trashp.tile([P, HALF], BF16, tag="trash")
                            nc.vector.scalar_tensor_tensor(
                                out=trash[:, 0 : sp1 - sp0],
                                in0=xt_ap(b, k, h * HALF + sp0, h * HALF + sp1),
                                scalar=1.0,
                                in1=wb[:, sp0:sp1],
                                op0=Mult,
                                op1=Mult,
                                accum_out=num_tiles[b][:, k, kslot : kslot + 1],
                            )

            def emit_out(b):
                nc.sync.dma_start(out=num_d.ap()[b], in_=num_tiles.pop(b)[:, :, :])
                nc.sync.dma_start(
                    out=den_d.ap()[b : b + 1, :], in_=den_tiles.pop(b)[0:1, :]
                )

            for i, (b, h) in enumerate(halves):
                last = i == NH - 1
                if i == 0:
                    # wt in m-major layout [P, MC, KC, 128]; the m=0 slice is
                    # its own DMA so the first MM block only waits for 128KB
                    wt_sb = singles.tile([P, MC, KC, P], BF16)
                    nc.sync.dma_start(out=wt_sb[:, 0], in_=wt_d.ap()[:, 0])
                    load_xt(0)
                    nc.sync.dma_start(out=wt_sb[:, 1:], in_=wt_d.ap()[:, 1:])
                    bc_sb = singles.tile([P, 2 * MC], F32)
                    nc.sync.dma_start(out=bc_sb[:, :], in_=bc_d.ap())
                    env["bc_sb"] = bc_sb
                    ctxr_sb = singles.tile([P, 2 * P], BF16)
                    nc.sync.dma_start(out=ctxr_sb[:, :], in_=ctxr_d.ap())
                    env["ctxr_sb"] = ctxr_sb
                    ones = singles.tile([P, P], BF16)
                    nc.gpsimd.memset(ones[:, :], 1.0)
                    env["ones"] = ones
                    # batch 0 remaining segments
                    segs = xt_segs[0]
                    for si, (c0, c1) in ((1, (512, 1024)), (2, (1024, 2048)), (3, (2048, 4096))):
                        for k in range(KC):
                            nc.sync.dma_start(
                                out=segs[si][0][:, k, :], in_=xt_d.ap()[0, k, :, c0:c1]
                            )
                if h == 0:
                    load_batch_state(b)

                act_sb = actpool.tile([P, MC, GPH, SG], BF16, tag="act")
                act_tiles[i] = act_sb

                for glp in range(2):
                    for m in range(MC):
                        ps = actps.tile([P, 2, SG], F32, tag="ps")
                        for j in range(2):
                            gl = glp * 2 + j
                            cc0 = h * HALF + gl * SG
                            for k in range(KC):
                                nc.tensor.matmul(
                                    ps[:, j, :],
                                    lhsT=wt_sb[:, m, k, :],
                                    rhs=xt_ap(b, k, cc0, cc0 + SG),
                                    start=(k == 0),
                                    stop=(k == KC - 1),
                                )
                        nc.scalar.activation(
                            out=act_sb[:, m, glp * 2 : (glp + 1) * 2, :],
                            in_=ps[:, :, :],
                            func=Tanh,
                            bias=bc_sb[:, m : m + 1],
                        )

                        # ---- interleave slots ----
                        if not last:
                            if glp == 0:
                                if m == 0 and i >= 1:
                                    emit_scores(i - 1, 0)
                                    emit_scores(i - 1, 1)
                                elif m == 1 and i >= 1:
                                    emit_scores(i - 1, 2)
                                    emit_scores(i - 1, 3)
                                elif m == 2 and i >= 1:
                                    emit_pool(i - 1)
                                    wb_tiles.pop(i - 1)
                                    act_tiles.pop(i - 1)
                                    if halves[i - 1][1] == NXT - 1:
                                        emit_out(halves[i - 1][0])
                                elif m == 3:
                                    if h == 0 and b + 1 < BPC:
                                        load_xt(b + 1)
                            else:
                                if m == 0:
                                    env["t0"] = emit_ts(i, 0, "ts0")
                                elif m == 1:
                                    t1 = emit_ts(i, 1, "ts1")
                                    s01 = saddp.tile(
                                        [P, HALF], BF16, tag="sadd", name="s01_t"
                                    )
                                    sadd_tiles[i] = s01
                                    nc.vector.tensor_tensor(
                                        out=s01[:, :], in0=env["t0"][:, :],
                                        in1=t1[:, :], op=Add,
                                    )
                        else:
                            if glp == 0:
                                if m == 0:
                                    emit_scores(i - 1, 0)
                                    emit_scores(i - 1, 1)
                                elif m == 1:
                                    emit_scores(i - 1, 2)
                                    emit_scores(i - 1, 3)

                    if last:
                        # final half: per-glp score pipeline -> short drain
                        lt0 = emit_ts(i, 0, "lts0", cols=(glp * 2, glp * 2 + 2))
                        lt1 = emit_ts(i, 1, "lts1", cols=(glp * 2, glp * 2 + 2))
                        s01 = saddp.tile([P, 2 * SG], BF16, tag="lsadd")
                        nc.vector.tensor_tensor(
                            out=s01[:, :], in0=lt0[:, :], in1=lt1[:, :], op=Add
                        )
                        for j in range(2):
                            gl = glp * 2 + j
                            scp = scps.tile([P, SG], F32, tag="scp", name="scp_t")
                            csl = slice(j * SG, (j + 1) * SG)
                            nc.tensor.matmul(
                                scp[:, :], lhsT=env["ones"][:, :], rhs=s01[:, csl],
                                start=True, stop=False,
                            )
                            nc.tensor.matmul(
                                scp[:, :], lhsT=env["ctxr_sb"][:, 0:P],
                                rhs=act_sb[:, 2, gl, :], start=False, stop=False,
                            )
                            nc.tensor.matmul(
                                scp[:, :], lhsT=env["ctxr_sb"][:, P : 2 * P],
                                rhs=act_sb[:, 3, gl, :], start=False, stop=True,
                            )
                            ef = efullp.tile([P, SG], BF16, tag="ef", name="ef_t")
                            nc.scalar.activation(out=ef[:, :], in_=scp[:, :], func=Exp)
                            if glp == 0 and j == 0:
                                wb_tiles[i] = wbcp.tile(
                                    [P, HALF], BF16, tag="wb", name="wb_t"
                                )
                            wb = wb_tiles[i]
                            ssl = slice(h * HALF + gl * SG, h * HALF + (gl + 1) * SG)
                            nc.vector.scalar_tensor_tensor(
                                out=wb[:, gl * SG : (gl + 1) * SG],
                                in0=ef[:, :],
                                scalar=-1.0,
                                in1=mask_tiles[b][:, ssl],
                                op0=Add,
                                op1=Mult,
                                accum_out=den_tiles[b][
                                    :, h * GPH + gl : h * GPH + gl + 1
                                ],
                            )
                        if glp == 0:
                            # previous half's pooling now that the drain
                            # chain's DVE ops are already queued ahead of it;
                            # its Act-side accums are deferred past the glp1
                            # tanh blocks to avoid head-of-line blocking
                            env["defer"] = []
                            emit_pool(i - 1, split=True, defer=env["defer"])
                            wb_tiles.pop(i - 1)
                            act_tiles.pop(i - 1)
                            emit_pool(
                                i, slot=1, cols=(0, HALF // 2),
                                split=True, defer=env["defer"],
                            )
                        else:
                            # flush deferred Act accums after the final exps
                            for fn in env.pop("defer"):
                                fn()
                            # final glp pool all-DVE (Act runs the deferred
                            # accums concurrently)
                            emit_pool(i, slot=2, cols=(HALF // 2, HALF))

            emit_out(BPC - 1)

    nc.compile()
    return nc


_NC_CACHE = {}


def _get_nc():
    if "nc" not in _NC_CACHE:
        _NC_CACHE["nc"] = build()
    return _NC_CACHE["nc"]


def kernel(inputs, mask, W, b, context):
    X = np.asarray(inputs, dtype=np.float32)
    mask = np.asarray(mask)
    W = np.asarray(W, dtype=np.float32)
    b = np.asarray(b, dtype=np.float32)
    context = np.asarray(context, dtype=np.float32)

    nc = _get_nc()

    xt_full = np.ascontiguousarray(X.transpose(0, 2, 1)).reshape(B, KC, P, S).astype(BF)
    # wt[p, m, k, j] = W[128m+j, 128k+p]
    wt = np.ascontiguousarray(
        W.reshape(MC, P, KC, P).transpose(3, 0, 2, 1)
    ).astype(BF)
    bc = np.concatenate(
        [b.reshape(MC, P).T, context.reshape(MC, P).T], axis=1
    ).astype(np.float32)
    bc = np.ascontiguousarray(bc)
    # ctxr[p, m*128+j] = context[128*(m+2)+p], replicated over j
    cr = context.reshape(MC, P)
    ctxr = np.ascontiguousarray(
        np.broadcast_to(
            cr[2:4, :, None], (2, P, P)
        ).transpose(1, 0, 2).reshape(P, 2 * P)
    ).astype(BF)
    # mask row-replicated across 128 partitions
    mask_rep = np.ascontiguousarray(
        np.broadcast_to(mask.astype(np.float32)[:, None, :], (B, P, S))
    ).astype(BF)

    in_maps = []
    for c in range(N_CORES):
        in_maps.append(
            {
                "xt": xt_full[c * BPC : (c + 1) * BPC],
                "wt": wt,
                "bc": bc,
                "ctxr": ctxr,
                "mask": mask_rep[c * BPC : (c + 1) * BPC],
            }
        )

    res = run_bass_kernel_spmd(
        nc, in_maps, core_ids=list(range(N_CORES)), trace=TRACE, tmpdir=TRACE_DIR
    )
    LAST["exec_time_ns"] = res.exec_time_ns
    LAST["result"] = res

    # host-side correction for the w = (exp(s)-1)*mask + 1 rewrite:
    # num += sum_s X[b,s,:], den += S
    xsum = X.astype(BF).astype(np.float32).sum(axis=1)  # [B, H]

    out = np.empty((B, H), np.float32)
    for c in range(N_CORES):
        num = res.results[c]["num"].sum(axis=3)  # [BPC, 128, KC]
        den = res.results[c]["den"].sum(axis=1) + float(S)  # [BPC]
        # num[b, p, k] -> out[b, k*128+p]
        numf = num.transpose(0, 2, 1).reshape(BPC, H) + xsum[c * BPC : (c + 1) * BPC]
        out[c * BPC : (c + 1) * BPC] = numf / den[:, None]
    return out
